# revision 24
# baseline (speedup 1.0000x reference)
"""BigBird block-sparse attention TRN2 kernel (8 NeuronCores, SPMD).

Sharding: core c handles batch b=c//2 and head-half hh=c%2 (8 of 16 heads,
feature slice hh*512..+512). Per core:
  Phase 1: QKV projection (q.T/k.T feature-major, v natural) -> DRAM.
  Phase 2: block-sparse attention. Middle query blocks (2..61) in strips of
    4 blocks, scores computed transposed (keys on partitions) so softmax'd
    probabilities feed P@V directly with no on-chip transpose. Masking of
    union-padded key blocks happens inside the QK matmul via 4 augmented
    contraction rows (one-hot on q side, -30000 ban pattern on k side).
    Denominator comes from an appended ones-column on V. Edge blocks
    (0,1,62,63) ship exp(scores) to the host, which does mask/normalize/PV
    (tiny FLOPs, avoids transposes for full-attention rows).
Host: normalizes middle ctx by denominator, computes edge PV, transposes
  feature-major ctx back to [S, F] and reassembles the full output.
"""
import sys

if "/opt/trn_rl_repo" not in sys.path:
    sys.path.insert(0, "/opt/trn_rl_repo")

import numpy as np

import concourse.bacc as bacc
import concourse.bass as bass
import concourse.tile as tile
from concourse import mybir
from concourse.bass_utils import run_bass_kernel_spmd

F32 = mybir.dt.float32
F32R = mybir.dt.float32r

B, S, H, HS, D, BLK = 4, 4096, 16, 1024, 64, 64
NB = S // BLK            # 64 key/query blocks
HPC = 8                  # heads per core
FPC = HPC * D            # 512 features per core
NKC = HS // 128          # 8 contraction chunks in phase 1
NSEQ = 8                 # phase-1 seq chunks of 512
NEG = -30000.0           # mask logit
NMID = 15                # middle strips of 4 blocks (blocks 2..61)
KTW = S + BLK            # kT cols incl. block-0 dup at 4096:4160

_BUILT = None


def _build():
    nc = bacc.Bacc(None, target_bir_lowering=False)

    # ---- parameters ----
    # [p, n, c, s']: X[n*512+s', c*128+p] -- per-(n) loads are contiguous
    xt = nc.declare_dram_parameter("xt", [128, NSEQ, NKC, 512], F32R, False)
    # [p, c, f]: W.T[c*128+p, f]
    wqt = nc.declare_dram_parameter("wqt", [128, NKC, FPC], F32R, False)
    wkt = nc.declare_dram_parameter("wkt", [128, NKC, FPC], F32R, False)
    wvt = nc.declare_dram_parameter("wvt", [128, NKC, FPC], F32R, False)
    bq = nc.declare_dram_parameter("bq", [128, 4], F32, False)       # pre-scaled by 1/8
    bk = nc.declare_dram_parameter("bk", [128, 4], F32, False)
    bv = nc.declare_dram_parameter("bv", [FPC], F32, False)
    onehot = nc.declare_dram_parameter("onehot", [8, S], F32R, False)
    ones = nc.declare_dram_parameter("ones", [128, 32, 1], F32R, False)
    kaug = nc.declare_dram_parameter("kaug", [8, KTW], F32R, False)

    ctxt = nc.declare_dram_parameter("ctxt", [HPC * 65, S], F32, True)
    pe1 = nc.declare_dram_parameter("pe1", [HPC * 128, S], F32, True)
    pe2 = nc.declare_dram_parameter("pe2", [HPC * 128, 6 * BLK], F32, True)
    vout = nc.declare_dram_parameter("vout", [S, FPC], F32R, True)   # v, host PV

    # ---- internal DRAM ----
    qt_d = nc.dram_tensor("qt_d", [FPC, S], F32R)
    kt_d = nc.dram_tensor("kt_d", [FPC, S], F32R)

    with tile.TileContext(nc) as tc:
        with tc.tile_pool(name="vperm", bufs=1) as vp:
            vaus = []
            for vi in range(2):
                vt = vp.tile([128, 32, 65], F32R, tag=f"vau{vi}", name=f"vau{vi}")
                nc.scalar.dma_start(out=vt[:, :, 64:65], in_=ones[:])
                vaus.append(vt)
            _phase1(nc, tc, xt, wqt, wkt, wvt, bq, bk, bv, qt_d, kt_d, vout)
            # phase 2 reads qt_d/kt_d/vout via DRAM, which Tile does not track
            tc.strict_bb_all_engine_barrier()
            _phase2(nc, tc, qt_d, kt_d, vout, onehot, kaug, ctxt, pe1, pe2, vaus)
    nc.compile()
    return nc


def _phase1(nc, tc, xt, wqt, wkt, wvt, bq, bk, bv, qt_d, kt_d, vout):
    """q.T = 0.125*(Wq@X.T + bq), k.T, v. q.T/k.T -> [FPC, S] DRAM, v -> [S, FPC]."""
    with tc.tile_pool(name="p1_w", bufs=1) as wp, \
         tc.tile_pool(name="p1_x", bufs=2) as xp, \
         tc.tile_pool(name="p1_ev", bufs=4) as ep, \
         tc.tile_pool(name="p1_ps", bufs=4, space="PSUM") as pp:
        # weights resident: [128, NKC, FPC] per projection (k-chunk major)
        xtiles = {}
        xt0 = xp.tile([128, NKC, 512], F32R, tag="xt")
        nc.sync.dma_start(out=xt0[:], in_=xt[:, 0])
        xtiles = {0: xt0}
        wts = {}
        for name, w in (("q", wqt), ("k", wkt), ("v", wvt)):
            t = wp.tile([128, NKC, FPC], F32R, tag=f"w{name}")
            nc.sync.dma_start(out=t[:], in_=w[:])
            wts[name] = t
        # biases: q/k as [128, 4] per-partition scalars; v broadcast [128, FPC]
        bqt = wp.tile([128, 4], F32, tag="bqt")
        bkt = wp.tile([128, 4], F32, tag="bkt")
        nc.sync.dma_start(out=bqt[:], in_=bq[:])
        nc.sync.dma_start(out=bkt[:], in_=bk[:])
        bvt = wp.tile([128, FPC], F32, tag="bvt")
        bv_ap = bv.ap()
        nc.sync.dma_start(
            out=bvt[:],
            in_=bass.AP(tensor=bv_ap.tensor, offset=bv_ap.offset,
                        ap=[[0, 128]] + bv_ap.ap),
        )

        for n in range(NSEQ):
            if n in xtiles:
                xtile = xtiles.pop(n)
            else:
                xtile = xp.tile([128, NKC, 512], F32R, tag="xt")
                nc.sync.dma_start(out=xtile[:], in_=xt[:, n])
            # q.T / k.T m-tiles: out rows = features
            for name, dst, bt, scl in (("q", qt_d, bqt, 0.125), ("k", kt_d, bkt, 1.0)):
                for m in range(4):
                    ps = pp.tile([128, 512], F32, tag="ps")
                    for kc in range(NKC):
                        nc.tensor.matmul(
                            ps[:],
                            wts[name][:, kc, m * 128:(m + 1) * 128],
                            xtile[:, kc, :],
                            start=(kc == 0), stop=(kc == NKC - 1),
                        )
                    ev = ep.tile([128, 512], F32R, tag="ev")
                    nc.scalar.activation(
                        ev[:], ps[:], mybir.ActivationFunctionType.Identity,
                        bias=bt[:, m:m + 1], scale=scl,
                    )
                    nc.scalar.dma_start(
                        out=dst[m * 128:(m + 1) * 128, n * 512:(n + 1) * 512],
                        in_=ev[:],
                    )
            # v: natural [seq, feat]
            for sm in range(4):
                ps = pp.tile([128, 512], F32, tag="ps")
                for kc in range(NKC):
                    nc.tensor.matmul(
                        ps[:],
                        xtile[:, kc, sm * 128:(sm + 1) * 128],
                        wts["v"][:, kc, :],
                        start=(kc == 0), stop=(kc == NKC - 1),
                    )
                ev = ep.tile([128, 512], F32R, tag="ev")
                nc.vector.tensor_add(ev[:], ps[:], bvt[:])
                nc.scalar.dma_start(
                    out=vout[n * 512 + sm * 128: n * 512 + (sm + 1) * 128, :],
                    in_=ev[:],
                )


def _phase2(nc, tc, qt_d, kt_d, vout, onehot, kaug, ctxt, pe1, pe2, vaus):
    with tc.tile_pool(name="p2_hd", bufs=1) as hp, \
         tc.tile_pool(name="p2_p", bufs=3) as pxp, \
         tc.tile_pool(name="p2_ev", bufs=1) as evp, \
         tc.tile_pool(name="p2_qk", bufs=3, space="PSUM") as qkp, \
         tc.tile_pool(name="p2_sm", bufs=2, space="PSUM") as smp:
        def loads(h):
            ktg = hp.tile([72, KTW], F32R, tag=f"ktg{h % 2}")
            nc.sync.dma_start(out=ktg[0:64, 0:S], in_=kt_d[h * 64:(h + 1) * 64, :])
            nc.sync.dma_start(out=ktg[0:64, S:KTW], in_=kt_d[h * 64:(h + 1) * 64, 0:BLK])
            nc.sync.dma_start(out=ktg[64:72, :], in_=kaug[:])
            qtg = hp.tile([72, S], F32R, tag=f"qtg{h % 2}")
            nc.sync.dma_start(out=qtg[0:64, :], in_=qt_d[h * 64:(h + 1) * 64, :])
            nc.sync.dma_start(out=qtg[64:72, :], in_=onehot[:])
            vau = vaus[h % 2]
            nc.scalar.dma_start(out=vau[0:64, 0, 0:64], in_=vout[S - BLK:S, h * 64:(h + 1) * 64])
            nc.scalar.dma_start(out=vau[64:128, 0, 0:64], in_=vout[0:BLK, h * 64:(h + 1) * 64])
            nc.scalar.dma_start(
                out=vau[:, 1:32, 0:64],
                in_=vout[BLK:BLK + 31 * 128, h * 64:(h + 1) * 64].rearrange(
                    "(c p) d -> p c d", p=128),
            )
            qe = hp.tile([64, 256], F32R, tag=f"qe{h % 2}")
            for i, qc in enumerate((0, 63, 1, 62)):
                nc.vector.tensor_copy(qe[:, i * 64:(i + 1) * 64],
                                      qtg[0:64, qc * BLK:(qc + 1) * BLK])
            ctx_acc = evp.tile([65, NMID * 256], F32, tag=f"ctx{h % 2}")
            return ktg, qtg, vau, qe, ctx_acc

        def strip(hs, s):
            ktg, qtg, vau, qe, ctx_acc = hs
            w = 4 * s + 2
            q0 = w * BLK
            sps = qkp.tile([128, 4, 256], F32, tag="qk")
            nc.tensor.matmul(sps[:, 0, :], ktg[:, S - BLK:S + BLK],
                             qtg[:, q0:q0 + 256], start=True, stop=True)
            for c in range(3):
                col = (w - 1 + 2 * c) * BLK
                nc.tensor.matmul(sps[:, 1 + c, :], ktg[:, col:col + 128],
                                 qtg[:, q0:q0 + 256], start=True, stop=True)
            pt = pxp.tile([128, 4, 256], F32R, tag="pmid")
            nc.scalar.activation(pt[:], sps[:], mybir.ActivationFunctionType.Exp)
            cps = smp.tile([65, 256], F32, tag="sm")
            nc.tensor.matmul(cps[:], vau[:, 0, :], pt[:, 0, :],
                             start=True, stop=False)
            for c in range(3):
                nc.tensor.matmul(cps[:], vau[:, 1 + 2 * s + c, :], pt[:, 1 + c, :],
                                 start=False, stop=(c == 2))
            nc.vector.tensor_copy(ctx_acc[:, s * 256:(s + 1) * 256], cps[:])

        def edges(h, hs):
            ktg, qtg, vau, qe, ctx_acc = hs
            nc.gpsimd.dma_start(
                out=ctxt[h * 65:(h + 1) * 65, 2 * BLK:62 * BLK], in_=ctx_acc[:]
            )
            pev = evp.tile([128, S], F32, tag=f"pe1{h % 2}")
            for c in range(8):
                eps = smp.tile([128, 512], F32, tag="sm")
                nc.tensor.matmul(eps[:], qe[:, 0:128],
                                 ktg[0:64, c * 512:(c + 1) * 512],
                                 start=True, stop=True)
                nc.scalar.activation(pev[:, c * 512:(c + 1) * 512], eps[:],
                                     mybir.ActivationFunctionType.Exp)
            nc.gpsimd.dma_start(out=pe1[h * 128:(h + 1) * 128, :], in_=pev[:])
            e2ps = smp.tile([128, 6 * BLK], F32, tag="sm")
            nc.tensor.matmul(e2ps[:, 0:192], qe[:, 128:256], ktg[0:64, 0:192],
                             start=True, stop=True)
            nc.tensor.matmul(e2ps[:, 192:384], qe[:, 128:256],
                             ktg[0:64, (NB - 3) * BLK:S],
                             start=True, stop=True)
            e2ev = evp.tile([128, 6 * BLK], F32, tag=f"pe2{h % 2}")
            nc.scalar.activation(e2ev[:], e2ps[:], mybir.ActivationFunctionType.Exp)
            nc.gpsimd.dma_start(out=pe2[h * 128:(h + 1) * 128, :], in_=e2ev[:])

        for h in range(HPC):
            hs = loads(h)
            for s in range(NMID):
                strip(hs, s)
            edges(h, hs)


def _wshuf(W, fs):
    # [p, c, f]: W.T[c*128+p, f] for the feature slice
    wt = np.asarray(W, np.float32)[fs, :].T  # [HS, FPC]
    return np.ascontiguousarray(wt.reshape(NKC, 128, FPC).transpose(1, 0, 2))


def _host_inputs(hidden, Wq, bq, Wk, bk, Wv, bv, c):
    b, hh = c // 2, c % 2
    fs = slice(hh * FPC, (hh + 1) * FPC)
    X = np.asarray(hidden[b], np.float32)
    xt = np.ascontiguousarray(X.reshape(NSEQ, 512, NKC, 128).transpose(3, 0, 2, 1))
    onehot = np.zeros((8, S), np.float32)
    qb = np.arange(S) // BLK
    for r8 in range(8):
        onehot[r8, (qb % 8) == ((r8 + 2) % 8)] = 1.0
    kaug = np.zeros((8, KTW), np.float32)
    for s_ in range(NMID):
        g = s_ % 2
        for r in range(4):
            for jj in range(6):
                if jj < r or jj > r + 2:
                    c0 = (4 * s_ + 1 + jj) * BLK
                    kaug[4 * g + r, c0:c0 + BLK] = NEG
    return {
        "xt": xt,
        "wqt": _wshuf(Wq, fs),
        "wkt": _wshuf(Wk, fs),
        "wvt": _wshuf(Wv, fs),
        "bq": (bq[fs] * np.float32(0.125)).astype(np.float32).reshape(4, 128).T.copy(),
        "bk": bk[fs].astype(np.float32).reshape(4, 128).T.copy(),
        "bv": bv[fs].astype(np.float32),
        "onehot": onehot,
        "ones": np.ones((128, 32, 1), np.float32),
        "kaug": kaug,
    }


def _host_finish(res_c):
    """Per-core host post-processing -> [S, FPC] output slice."""
    ctxt = res_c["ctxt"]
    p1 = res_c["pe1"]
    p2 = res_c["pe2"]
    v = res_c["vout"]  # [S, FPC]
    out = np.empty((S, FPC), np.float32)
    for h in range(HPC):
        vh = v[:, h * 64:(h + 1) * 64]
        # middle blocks 2..61
        num = ctxt[h * 65:h * 65 + 64, 2 * BLK:62 * BLK]
        den = ctxt[h * 65 + 64, 2 * BLK:62 * BLK]
        out[2 * BLK:62 * BLK, h * 64:(h + 1) * 64] = (num / den).T
        # E1: blocks 0, 63 (full attention)
        P = p1[h * 128:(h + 1) * 128, :]
        C = (P / P.sum(1, keepdims=True)) @ vh
        out[0:BLK, h * 64:(h + 1) * 64] = C[0:64]
        out[S - BLK:S, h * 64:(h + 1) * 64] = C[64:128]
        # E2: blocks 1, 62; key cols = blocks {0,1,2} then {61,62,63}
        P = p2[h * 128:(h + 1) * 128, :].copy()
        P[0:64, 192:320] = 0.0    # block 1 bans blocks 61, 62
        P[64:128, 64:192] = 0.0   # block 62 bans blocks 1, 2
        vk = np.concatenate([vh[0:192], vh[(NB - 3) * BLK:]], 0)
        C = (P / P.sum(1, keepdims=True)) @ vk
        out[BLK:2 * BLK, h * 64:(h + 1) * 64] = C[0:64]
        out[62 * BLK:63 * BLK, h * 64:(h + 1) * 64] = C[64:128]
    return out


def _run(inputs, trace=False):
    global _BUILT
    if _BUILT is None:
        _BUILT = _build()
    core_ids = list(range(8))
    in_maps = [_host_inputs(**inputs, c=c) for c in core_ids]
    res = run_bass_kernel_spmd(_BUILT, in_maps, core_ids, trace=trace)
    out = np.empty((B, S, HS), np.float32)
    for c in core_ids:
        b, hh = c // 2, c % 2
        out[b, :, hh * FPC:(hh + 1) * FPC] = _host_finish(res.results[c])
    return out, res


def kernel(hidden_states, Wq, bq, Wk, bk, Wv, bv):
    inputs = dict(hidden=np.asarray(hidden_states), Wq=np.asarray(Wq),
                  bq=np.asarray(bq), Wk=np.asarray(Wk), bk=np.asarray(bk),
                  Wv=np.asarray(Wv), bv=np.asarray(bv))
    out, _ = _run(inputs, trace=False)
    return out


# revision 25
# speedup vs baseline: 1.1121x; 1.1121x over previous
"""BigBird block-sparse attention TRN2 kernel (8 NeuronCores, SPMD).

Sharding: core c handles batch b=c//2 and head-half hh=c%2 (8 of 16 heads,
feature slice hh*512..+512). Per core:
  Phase 1: QKV projection (q.T/k.T feature-major, v natural) -> DRAM.
  Phase 2: block-sparse attention. Middle query blocks (2..61) in strips of
    4 blocks, scores computed transposed (keys on partitions) so softmax'd
    probabilities feed P@V directly with no on-chip transpose. Masking of
    union-padded key blocks happens inside the QK matmul via 4 augmented
    contraction rows (one-hot on q side, -30000 ban pattern on k side).
    Denominator comes from an appended ones-column on V. Edge blocks
    (0,1,62,63) ship exp(scores) to the host, which does mask/normalize/PV
    (tiny FLOPs, avoids transposes for full-attention rows).
Host: normalizes middle ctx by denominator, computes edge PV, transposes
  feature-major ctx back to [S, F] and reassembles the full output.
"""
import sys

if "/opt/trn_rl_repo" not in sys.path:
    sys.path.insert(0, "/opt/trn_rl_repo")

import numpy as np

import concourse.bacc as bacc
import concourse.bass as bass
import concourse.tile as tile
from concourse import mybir
from concourse.bass_utils import run_bass_kernel_spmd

F32 = mybir.dt.float32
F32R = mybir.dt.float32r

B, S, H, HS, D, BLK = 4, 4096, 16, 1024, 64, 64
NB = S // BLK            # 64 key/query blocks
HPC = 8                  # heads per core
FPC = HPC * D            # 512 features per core
NKC = HS // 128          # 8 contraction chunks in phase 1
NSEQ = 8                 # phase-1 seq chunks of 512
NEG = -30000.0           # mask logit
NMID = 15                # middle strips of 4 blocks (blocks 2..61)
KTW = S + BLK            # kT cols incl. block-0 dup at 4096:4160

_BUILT = None


def _build():
    nc = bacc.Bacc(None, target_bir_lowering=False)

    # ---- parameters ----
    # [p, n, c, s']: X[n*512+s', c*128+p] -- per-(n) loads are contiguous
    xt = nc.declare_dram_parameter("xt", [128, NSEQ, NKC, 512], F32R, False)
    # [p, c, f]: W.T[c*128+p, f]
    wqt = nc.declare_dram_parameter("wqt", [128, NKC, FPC], F32R, False)
    wkt = nc.declare_dram_parameter("wkt", [128, NKC, FPC], F32R, False)
    wvt = nc.declare_dram_parameter("wvt", [128, NKC, FPC], F32R, False)
    bq = nc.declare_dram_parameter("bq", [128, 4], F32, False)       # pre-scaled by 1/8
    bk = nc.declare_dram_parameter("bk", [128, 4], F32, False)
    bv = nc.declare_dram_parameter("bv", [FPC], F32, False)
    onehot = nc.declare_dram_parameter("onehot", [8, S], F32R, False)
    ones = nc.declare_dram_parameter("ones", [128, 32, 1], F32R, False)
    kaug = nc.declare_dram_parameter("kaug", [8, KTW], F32R, False)

    ctxt = nc.declare_dram_parameter("ctxt", [HPC * 65, S], F32, True)
    pe1 = nc.declare_dram_parameter("pe1", [HPC * 128, S], F32, True)
    pe2 = nc.declare_dram_parameter("pe2", [HPC * 128, 6 * BLK], F32, True)
    vout = nc.declare_dram_parameter("vout", [S, FPC], F32R, True)   # v, host PV

    # ---- internal DRAM ----
    qt_d = nc.dram_tensor("qt_d", [FPC, S], F32R)
    kt_d = nc.dram_tensor("kt_d", [FPC, S], F32R)

    with tile.TileContext(nc) as tc:
        with tc.tile_pool(name="vperm", bufs=1) as vp:
            vaus = []
            for vi in range(2):
                vt = vp.tile([128, 32, 65], F32R, tag=f"vau{vi}", name=f"vau{vi}")
                nc.scalar.dma_start(out=vt[:, :, 64:65], in_=ones[:])
                vaus.append(vt)
            _phase1(nc, tc, xt, wqt, wkt, wvt, bq, bk, bv, qt_d, kt_d, vout)
            # phase 2 reads qt_d/kt_d/vout via DRAM, which Tile does not track
            tc.strict_bb_all_engine_barrier()
            _phase2(nc, tc, qt_d, kt_d, vout, onehot, kaug, ctxt, pe1, pe2, vaus)
    nc.compile()
    return nc


def _phase1(nc, tc, xt, wqt, wkt, wvt, bq, bk, bv, qt_d, kt_d, vout):
    """q.T = 0.125*(Wq@X.T + bq), k.T, v. q.T/k.T -> [FPC, S] DRAM, v -> [S, FPC]."""
    with tc.tile_pool(name="p1_w", bufs=1) as wp, \
         tc.tile_pool(name="p1_x", bufs=2) as xp, \
         tc.tile_pool(name="p1_ev", bufs=4) as ep, \
         tc.tile_pool(name="p1_ps", bufs=4, space="PSUM") as pp:
        # weights resident: [128, NKC, FPC] per projection (k-chunk major)
        xtiles = {}
        xt0 = xp.tile([128, NKC, 512], F32R, tag="xt")
        nc.sync.dma_start(out=xt0[:], in_=xt[:, 0])
        xtiles = {0: xt0}
        wts = {}
        for name, w in (("q", wqt), ("k", wkt), ("v", wvt)):
            t = wp.tile([128, NKC, FPC], F32R, tag=f"w{name}")
            nc.sync.dma_start(out=t[:], in_=w[:])
            wts[name] = t
        # biases: q/k as [128, 4] per-partition scalars; v broadcast [128, FPC]
        bqt = wp.tile([128, 4], F32, tag="bqt")
        bkt = wp.tile([128, 4], F32, tag="bkt")
        nc.sync.dma_start(out=bqt[:], in_=bq[:])
        nc.sync.dma_start(out=bkt[:], in_=bk[:])
        bvt = wp.tile([128, FPC], F32, tag="bvt")
        bv_ap = bv.ap()
        nc.sync.dma_start(
            out=bvt[:],
            in_=bass.AP(tensor=bv_ap.tensor, offset=bv_ap.offset,
                        ap=[[0, 128]] + bv_ap.ap),
        )

        for n in range(NSEQ):
            if n in xtiles:
                xtile = xtiles.pop(n)
            else:
                xtile = xp.tile([128, NKC, 512], F32R, tag="xt")
                nc.sync.dma_start(out=xtile[:], in_=xt[:, n])
            # q.T / k.T m-tiles: out rows = features
            for name, dst, bt, scl in (("q", qt_d, bqt, 0.125), ("k", kt_d, bkt, 1.0)):
                for m in range(4):
                    ps = pp.tile([128, 512], F32, tag="ps")
                    for kc in range(NKC):
                        nc.tensor.matmul(
                            ps[:],
                            wts[name][:, kc, m * 128:(m + 1) * 128],
                            xtile[:, kc, :],
                            start=(kc == 0), stop=(kc == NKC - 1),
                        )
                    ev = ep.tile([128, 512], F32R, tag="ev")
                    nc.scalar.activation(
                        ev[:], ps[:], mybir.ActivationFunctionType.Identity,
                        bias=bt[:, m:m + 1], scale=scl,
                    )
                    nc.scalar.dma_start(
                        out=dst[m * 128:(m + 1) * 128, n * 512:(n + 1) * 512],
                        in_=ev[:],
                    )
            # v: natural [seq, feat]
            for sm in range(4):
                ps = pp.tile([128, 512], F32, tag="ps")
                for kc in range(NKC):
                    nc.tensor.matmul(
                        ps[:],
                        xtile[:, kc, sm * 128:(sm + 1) * 128],
                        wts["v"][:, kc, :],
                        start=(kc == 0), stop=(kc == NKC - 1),
                    )
                ev = ep.tile([128, 512], F32R, tag="ev")
                nc.vector.tensor_add(ev[:], ps[:], bvt[:])
                nc.scalar.dma_start(
                    out=vout[n * 512 + sm * 128: n * 512 + (sm + 1) * 128, :],
                    in_=ev[:],
                )


def _phase2(nc, tc, qt_d, kt_d, vout, onehot, kaug, ctxt, pe1, pe2, vaus):
    with tc.tile_pool(name="p2_hd", bufs=1) as hp, \
         tc.tile_pool(name="p2_p", bufs=3) as pxp, \
         tc.tile_pool(name="p2_ev", bufs=1) as evp, \
         tc.tile_pool(name="p2_qk", bufs=3, space="PSUM") as qkp, \
         tc.tile_pool(name="p2_sm", bufs=2, space="PSUM") as smp:
        def loads(h):
            ktg = hp.tile([72, KTW], F32R, tag=f"ktg{h % 2}")
            nc.sync.dma_start(out=ktg[0:64, 0:S], in_=kt_d[h * 64:(h + 1) * 64, :])
            nc.sync.dma_start(out=ktg[0:64, S:KTW], in_=kt_d[h * 64:(h + 1) * 64, 0:BLK])
            nc.sync.dma_start(out=ktg[64:72, :], in_=kaug[:])
            qtg = hp.tile([72, S], F32R, tag=f"qtg{h % 2}")
            nc.sync.dma_start(out=qtg[0:64, :], in_=qt_d[h * 64:(h + 1) * 64, :])
            nc.sync.dma_start(out=qtg[64:72, :], in_=onehot[:])
            vau = vaus[h % 2]
            nc.gpsimd.dma_start(out=vau[0:64, 0, 0:64], in_=vout[S - BLK:S, h * 64:(h + 1) * 64])
            nc.gpsimd.dma_start(out=vau[64:128, 0, 0:64], in_=vout[0:BLK, h * 64:(h + 1) * 64])
            nc.gpsimd.dma_start(
                out=vau[:, 1:32, 0:64],
                in_=vout[BLK:BLK + 31 * 128, h * 64:(h + 1) * 64].rearrange(
                    "(c p) d -> p c d", p=128),
            )
            qe = hp.tile([64, 256], F32R, tag=f"qe{h % 2}")
            for i, qc in enumerate((0, 63, 1, 62)):
                nc.vector.tensor_copy(qe[:, i * 64:(i + 1) * 64],
                                      qtg[0:64, qc * BLK:(qc + 1) * BLK])
            ctx_acc = evp.tile([65, NMID * 256], F32, tag=f"ctx{h % 2}")
            return ktg, qtg, vau, qe, ctx_acc

        def strip(hs, s):
            ktg, qtg, vau, qe, ctx_acc = hs
            w = 4 * s + 2
            q0 = w * BLK
            sps = qkp.tile([128, 4, 256], F32, tag="qk")
            nc.tensor.matmul(sps[:, 0, :], ktg[:, S - BLK:S + BLK],
                             qtg[:, q0:q0 + 256], start=True, stop=True)
            for c in range(3):
                col = (w - 1 + 2 * c) * BLK
                nc.tensor.matmul(sps[:, 1 + c, :], ktg[:, col:col + 128],
                                 qtg[:, q0:q0 + 256], start=True, stop=True)
            pt = pxp.tile([128, 4, 256], F32R, tag="pmid")
            nc.scalar.activation(pt[:], sps[:], mybir.ActivationFunctionType.Exp)
            cps = smp.tile([65, 256], F32, tag="sm")
            nc.tensor.matmul(cps[:], vau[:, 0, :], pt[:, 0, :],
                             start=True, stop=False)
            for c in range(3):
                nc.tensor.matmul(cps[:], vau[:, 1 + 2 * s + c, :], pt[:, 1 + c, :],
                                 start=False, stop=(c == 2))
            nc.vector.tensor_copy(ctx_acc[:, s * 256:(s + 1) * 256], cps[:])

        def edges(h, hs):
            ktg, qtg, vau, qe, ctx_acc = hs
            nc.gpsimd.dma_start(
                out=ctxt[h * 65:(h + 1) * 65, 2 * BLK:62 * BLK], in_=ctx_acc[:]
            )
            pev = evp.tile([128, S], F32, tag=f"pe1{h % 2}")
            for c in range(8):
                eps = smp.tile([128, 512], F32, tag="sm")
                nc.tensor.matmul(eps[:], qe[:, 0:128],
                                 ktg[0:64, c * 512:(c + 1) * 512],
                                 start=True, stop=True)
                nc.scalar.activation(pev[:, c * 512:(c + 1) * 512], eps[:],
                                     mybir.ActivationFunctionType.Exp)
            nc.gpsimd.dma_start(out=pe1[h * 128:(h + 1) * 128, :], in_=pev[:])
            e2ps = smp.tile([128, 6 * BLK], F32, tag="sm")
            nc.tensor.matmul(e2ps[:, 0:192], qe[:, 128:256], ktg[0:64, 0:192],
                             start=True, stop=True)
            nc.tensor.matmul(e2ps[:, 192:384], qe[:, 128:256],
                             ktg[0:64, (NB - 3) * BLK:S],
                             start=True, stop=True)
            e2ev = evp.tile([128, 6 * BLK], F32, tag=f"pe2{h % 2}")
            nc.scalar.activation(e2ev[:], e2ps[:], mybir.ActivationFunctionType.Exp)
            nc.gpsimd.dma_start(out=pe2[h * 128:(h + 1) * 128, :], in_=e2ev[:])

        for h in range(HPC):
            hs = loads(h)
            for s in range(NMID):
                strip(hs, s)
            edges(h, hs)


def _wshuf(W, fs):
    # [p, c, f]: W.T[c*128+p, f] for the feature slice
    wt = np.asarray(W, np.float32)[fs, :].T  # [HS, FPC]
    return np.ascontiguousarray(wt.reshape(NKC, 128, FPC).transpose(1, 0, 2))


def _host_inputs(hidden, Wq, bq, Wk, bk, Wv, bv, c):
    b, hh = c // 2, c % 2
    fs = slice(hh * FPC, (hh + 1) * FPC)
    X = np.asarray(hidden[b], np.float32)
    xt = np.ascontiguousarray(X.reshape(NSEQ, 512, NKC, 128).transpose(3, 0, 2, 1))
    onehot = np.zeros((8, S), np.float32)
    qb = np.arange(S) // BLK
    for r8 in range(8):
        onehot[r8, (qb % 8) == ((r8 + 2) % 8)] = 1.0
    kaug = np.zeros((8, KTW), np.float32)
    for s_ in range(NMID):
        g = s_ % 2
        for r in range(4):
            for jj in range(6):
                if jj < r or jj > r + 2:
                    c0 = (4 * s_ + 1 + jj) * BLK
                    kaug[4 * g + r, c0:c0 + BLK] = NEG
    return {
        "xt": xt,
        "wqt": _wshuf(Wq, fs),
        "wkt": _wshuf(Wk, fs),
        "wvt": _wshuf(Wv, fs),
        "bq": (bq[fs] * np.float32(0.125)).astype(np.float32).reshape(4, 128).T.copy(),
        "bk": bk[fs].astype(np.float32).reshape(4, 128).T.copy(),
        "bv": bv[fs].astype(np.float32),
        "onehot": onehot,
        "ones": np.ones((128, 32, 1), np.float32),
        "kaug": kaug,
    }


def _host_finish(res_c):
    """Per-core host post-processing -> [S, FPC] output slice."""
    ctxt = res_c["ctxt"]
    p1 = res_c["pe1"]
    p2 = res_c["pe2"]
    v = res_c["vout"]  # [S, FPC]
    out = np.empty((S, FPC), np.float32)
    for h in range(HPC):
        vh = v[:, h * 64:(h + 1) * 64]
        # middle blocks 2..61
        num = ctxt[h * 65:h * 65 + 64, 2 * BLK:62 * BLK]
        den = ctxt[h * 65 + 64, 2 * BLK:62 * BLK]
        out[2 * BLK:62 * BLK, h * 64:(h + 1) * 64] = (num / den).T
        # E1: blocks 0, 63 (full attention)
        P = p1[h * 128:(h + 1) * 128, :]
        C = (P / P.sum(1, keepdims=True)) @ vh
        out[0:BLK, h * 64:(h + 1) * 64] = C[0:64]
        out[S - BLK:S, h * 64:(h + 1) * 64] = C[64:128]
        # E2: blocks 1, 62; key cols = blocks {0,1,2} then {61,62,63}
        P = p2[h * 128:(h + 1) * 128, :].copy()
        P[0:64, 192:320] = 0.0    # block 1 bans blocks 61, 62
        P[64:128, 64:192] = 0.0   # block 62 bans blocks 1, 2
        vk = np.concatenate([vh[0:192], vh[(NB - 3) * BLK:]], 0)
        C = (P / P.sum(1, keepdims=True)) @ vk
        out[BLK:2 * BLK, h * 64:(h + 1) * 64] = C[0:64]
        out[62 * BLK:63 * BLK, h * 64:(h + 1) * 64] = C[64:128]
    return out


def _run(inputs, trace=False):
    global _BUILT
    if _BUILT is None:
        _BUILT = _build()
    core_ids = list(range(8))
    in_maps = [_host_inputs(**inputs, c=c) for c in core_ids]
    res = run_bass_kernel_spmd(_BUILT, in_maps, core_ids, trace=trace)
    out = np.empty((B, S, HS), np.float32)
    for c in core_ids:
        b, hh = c // 2, c % 2
        out[b, :, hh * FPC:(hh + 1) * FPC] = _host_finish(res.results[c])
    return out, res


def kernel(hidden_states, Wq, bq, Wk, bk, Wv, bv):
    inputs = dict(hidden=np.asarray(hidden_states), Wq=np.asarray(Wq),
                  bq=np.asarray(bq), Wk=np.asarray(Wk), bk=np.asarray(bk),
                  Wv=np.asarray(Wv), bv=np.asarray(bv))
    out, _ = _run(inputs, trace=False)
    return out
